# revision 41
# baseline (speedup 1.0000x reference)
"""MLA (multi-head latent attention) forward, sharded over 8 TRN2 NeuronCores.

Tensor-parallel over heads (2 heads/core).  The q path folds rmsnorm into the
B-projection and fuses A@B into per-head weights (rmsnorm's per-token scale
commutes through the matmul).  The kv path instead exploits the low-rank
structure: each core computes the 512-dim latent kv_c (and rope'd k_pe input)
for its OWN 512-token shard only, the shards are AllGather'd in bf16, and
each core then applies its heads' 512->256 B-projections — much cheaper than
fusing through the 2048-dim hidden.  A second, tiny AllGather shares the
per-token inverse-rms scalars.

Phase 2 runs attention per (batch, 512-query block) with both heads' streams
interleaved so every engine has two independent chains: scores^T in PSUM,
exp on the scalar engine, denominator via a ones-matmul in fp32r (single
pass), fast-approx reciprocal, softmax normalization + wo matmuls deferred
into drain slots of later streams.  wo output is written bf16; the host sums
the 8 partial outputs (the "all-reduce after wo" of the sharding hint).

Matmuls run in bf16 with fp32 PSUM accumulation; softmax and statistics stay
fp32.  RoPE features are permuted to a split even/odd layout on both q and k
(host permutes the projection weight rows identically, dot products unchanged).
"""
import sys
from contextlib import ExitStack

sys.path.insert(0, "/opt/trn_rl_repo")

import numpy as np
import ml_dtypes

import concourse.mybir as mybir
from concourse import bacc
from concourse.tile import TileContext
from concourse.bass_utils import run_bass_kernel_spmd

BF16 = ml_dtypes.bfloat16
F32 = mybir.dt.float32
F32R = mybir.dt.float32r
BF = mybir.dt.bfloat16

B, S, H = 2, 2048, 2048
NH = 16
Q_LORA, KV_LORA = 1536, 512
D_NOPE, D_ROPE, D_V = 128, 64, 128
D_QK = D_NOPE + D_ROPE
SCALE = 1.0 / float(np.sqrt(D_QK))
EPS = 1e-6

N_CORES = 8
HPC = NH // N_CORES          # heads per core = 2
TOK = B * S                  # 4096
TOKS = TOK // N_CORES        # 512-token shard per core
KC = H // 128                # 16 contraction chunks over hidden features
NB = TOK // 512              # 8 token blocks of 512
NQ = 384                     # fused q features per core (2*128 nope + 128 pe)
CC2N = 4 * 128 * TOKS + 64 * TOKS   # bf16 elems per core in the kv gather


def _host_tables():
    inv = 1.0 / (10000.0 ** (np.arange(0, D_ROPE, 2, dtype=np.float32) / D_ROPE))
    t = np.arange(S, dtype=np.float32)
    f = np.outer(t, inv)                       # (S, 32)
    cos = np.tile(np.cos(f).T, (1, B))         # (32, TOK), tokens b-major
    sin = np.tile(np.sin(f).T, (1, B))
    csq1 = np.concatenate([cos, cos, sin, sin], axis=0)   # (128, TOK)
    csq2 = np.concatenate([sin, sin, cos, cos], axis=0)
    csk1 = np.concatenate([cos, sin], axis=0)             # (64, TOK)
    csk2 = np.concatenate([sin, cos], axis=0)
    return [np.ascontiguousarray(x).astype(BF16) for x in (csq1, csq2, csk1, csk2)]


def _host_prep(hidden_states, wq_a, q_norm_w, wq_b, wkv_a, kv_norm_w, wkv_b, wo):
    hid = np.ascontiguousarray(np.asarray(hidden_states, dtype=np.float32).reshape(TOK, H))
    hT_bf = np.ascontiguousarray(hid.T).astype(BF16)             # (H, TOK)

    wq_b_f = (np.asarray(wq_b) * np.asarray(q_norm_w)[None, :]).astype(np.float32)
    wkv_b_f = (np.asarray(wkv_b) * np.asarray(kv_norm_w)[None, :]).astype(np.float32)

    Wq = wq_b_f @ np.asarray(wq_a)                 # (NH*192, H)
    wkpe = np.asarray(wkv_a)[KV_LORA:]             # (64, H)

    ev = np.arange(0, D_ROPE, 2)
    od = np.arange(1, D_ROPE, 2)
    csq1, csq2, csk1, csk2 = _host_tables()

    wqaT = np.ascontiguousarray(np.asarray(wq_a).T).astype(BF16)
    wkvaT = np.ascontiguousarray(np.asarray(wkv_a)[:KV_LORA].T).astype(BF16)
    wkpeT = np.ascontiguousarray(
        np.concatenate([wkpe[ev], wkpe[od]], axis=0).T).astype(BF16)   # (H, 64)

    in_maps = []
    for c in range(N_CORES):
        h0, h1 = 2 * c, 2 * c + 1
        qh = [Wq[h * D_QK:(h + 1) * D_QK] for h in (h0, h1)]
        qpe0, qpe1 = qh[0][D_NOPE:], qh[1][D_NOPE:]
        W_q_all = np.concatenate([
            qh[0][:D_NOPE], qh[1][:D_NOPE],
            qpe0[ev], qpe1[ev], qpe0[od], qpe1[od],
        ], axis=0)                                               # (384, H)
        WqT = np.ascontiguousarray(W_q_all.T).astype(BF16)       # (H, 384)

        bh = [wkv_b_f[h * (D_NOPE + D_V):(h + 1) * (D_NOPE + D_V)] for h in (h0, h1)]
        WknT = np.ascontiguousarray(np.concatenate(
            [bh[0][:D_NOPE], bh[1][:D_NOPE]], axis=0).T).astype(BF16)   # (512, 256)
        WvT = np.ascontiguousarray(np.concatenate(
            [bh[0][D_NOPE:], bh[1][D_NOPE:]], axis=0).T).astype(BF16)   # (512, 256)

        wo_h = np.asarray(wo)[:, c * HPC * D_V:(c + 1) * HPC * D_V]   # (H, 256)
        woR = np.ascontiguousarray(wo_h.T).astype(BF16)          # (256, H)

        in_maps.append({
            "hT": hT_bf,
            "hTs": np.ascontiguousarray(hT_bf[:, c * TOKS:(c + 1) * TOKS]),
            "wqaT": wqaT,
            "wkvaT": wkvaT,
            "wkpeT": wkpeT,
            "WqT": WqT,
            "WknT": WknT,
            "WvT": WvT,
            "woR": woR,
            "csq1": csq1, "csq2": csq2, "csk1": csk1, "csk2": csk2,
        })
    return in_maps


def _build_program():
    nc = bacc.Bacc()

    hT = nc.dram_tensor("hT", [H, TOK], BF, kind="ExternalInput")
    hTs = nc.dram_tensor("hTs", [H, TOKS], BF, kind="ExternalInput")
    wqaT = nc.dram_tensor("wqaT", [H, Q_LORA], BF, kind="ExternalInput")
    wkvaT = nc.dram_tensor("wkvaT", [H, KV_LORA], BF, kind="ExternalInput")
    wkpeT = nc.dram_tensor("wkpeT", [H, 64], BF, kind="ExternalInput")
    WqT = nc.dram_tensor("WqT", [H, NQ], BF, kind="ExternalInput")
    WknT = nc.dram_tensor("WknT", [KV_LORA, HPC * D_NOPE], BF, kind="ExternalInput")
    WvT = nc.dram_tensor("WvT", [KV_LORA, HPC * D_V], BF, kind="ExternalInput")
    woR = nc.dram_tensor("woR", [HPC * D_V, H], BF, kind="ExternalInput")
    csq1d = nc.dram_tensor("csq1", [128, TOK], BF, kind="ExternalInput")
    csq2d = nc.dram_tensor("csq2", [128, TOK], BF, kind="ExternalInput")
    csk1d = nc.dram_tensor("csk1", [64, TOK], BF, kind="ExternalInput")
    csk2d = nc.dram_tensor("csk2", [64, TOK], BF, kind="ExternalInput")
    out = nc.dram_tensor("out", [TOK, H], BF, kind="ExternalOutput")

    AF = mybir.ActivationFunctionType
    OP = mybir.AluOpType

    with TileContext(nc) as tc, ExitStack() as top:
        if True:
            onesp = top.enter_context(tc.tile_pool(name="onesp", bufs=1))
            ccp = top.enter_context(tc.tile_pool(name="ccp", bufs=1, space="DRAM"))
            wqp = top.enter_context(tc.tile_pool(name="wqp", bufs=1))
            csp = top.enter_context(tc.tile_pool(name="csp", bufs=1))

            ones_col_f = onesp.tile([128, 1], F32)
            ones_col = onesp.tile([128, 1], F32R)
            eps_col = onesp.tile([128, 1], F32)
            eps_row = onesp.tile([1, 1], F32)
            nc.vector.memset(ones_col_f[:], 1.0)
            nc.vector.tensor_copy(ones_col[:], ones_col_f[:])
            nc.vector.memset(eps_col[:], EPS)
            nc.vector.memset(eps_row[:], EPS)
            cc1_in = ccp.tile([1, 2 * TOKS], F32)
            cc1_out = ccp.tile([N_CORES, 2 * TOKS], F32, addr_space="Shared")
            cc2_in = ccp.tile([1, CC2N], BF)
            cc2_out = ccp.tile([N_CORES, CC2N], BF, addr_space="Shared")

            # ---------------- phase 0: latent kv + rms scalars -------------
            with ExitStack() as p0stack:
                p0w = p0stack.enter_context(tc.tile_pool(name="p0w", bufs=1))
                p0ps = p0stack.enter_context(
                    tc.tile_pool(name="p0ps", bufs=1, space="PSUM"))
                p0qps = p0stack.enter_context(
                    tc.tile_pool(name="p0qps", bufs=2, space="PSUM"))
                p0sb = p0stack.enter_context(tc.tile_pool(name="p0sb", bufs=2))

                # spread startup loads over all three DMA-issuing queues
                hts_t = []
                for k in range(KC):
                    t = p0w.tile([128, TOKS], BF, tag=f"hts{k}", name=f"hts{k}")
                    eng = nc.sync if k % 2 == 0 else nc.scalar
                    eng.dma_start(t[:], hTs[k * 128:(k + 1) * 128, :])
                    hts_t.append(t)
                wkva_t = []
                for k in range(KC):
                    t = p0w.tile([128, KV_LORA], BF, tag=f"wkva{k}", name=f"wkva{k}")
                    nc.gpsimd.dma_start(t[:], wkvaT[k * 128:(k + 1) * 128, :])
                    wkva_t.append(t)
                wkpe_t = []
                for k in range(KC):
                    t = p0w.tile([128, 64], BF, tag=f"wkpe{k}", name=f"wkpe{k}")
                    nc.gpsimd.dma_start(t[:], wkpeT[k * 128:(k + 1) * 128, :])
                    wkpe_t.append(t)
                # wqa in three 512-column blocks so q-norm matmuls can start
                # before the whole 6.3MB lands
                wqa_t = []
                for k in range(KC):
                    t = p0w.tile([128, Q_LORA], BF, tag=f"wqa{k}", name=f"wqa{k}")
                    wqa_t.append(t)
                for nb3 in range(Q_LORA // 512):
                    for k in range(KC):
                        eng = (nc.sync, nc.scalar, nc.gpsimd)[k % 3]
                        eng.dma_start(
                            wqa_t[k][:, nb3 * 512:(nb3 + 1) * 512],
                            wqaT[k * 128:(k + 1) * 128, nb3 * 512:(nb3 + 1) * 512])

                # phase-1 weight prefetch behind the p0 loads
                wq_t = []
                for k in range(KC):
                    t = wqp.tile([128, NQ], BF, tag=f"wq{k}", name=f"wq{k}")
                    eng = (nc.sync, nc.scalar, nc.gpsimd)[k % 3]
                    eng.dma_start(t[:], WqT[k * 128:(k + 1) * 128, :])
                    wq_t.append(t)
                csq1_t = csp.tile([128, TOK], BF, tag="csq1", name="csq1")
                csq2_t = csp.tile([128, TOK], BF, tag="csq2", name="csq2")
                nc.sync.dma_start(csq1_t[:], csq1d[:])
                nc.scalar.dma_start(csq2_t[:], csq2d[:])
                csk1_t = csp.tile([64, TOK], BF, tag="csk1", name="csk1")
                csk2_t = csp.tile([64, TOK], BF, tag="csk2", name="csk2")
                nc.sync.dma_start(csk1_t[:], csk1d[:])
                nc.scalar.dma_start(csk2_t[:], csk2d[:])

                # ---- latent kv (lora-major) for own shard ----
                pkv = [p0ps.tile([128, TOKS], F32, tag=f"pkv{m}", name=f"pkv{m}")
                       for m in range(4)]
                pkpe = p0ps.tile([64, TOKS], F32, tag="pkpe", name="pkpe")
                for k in range(KC):
                    for m in range(4):
                        nc.tensor.matmul(
                            pkv[m][:],
                            lhsT=wkva_t[k][:, m * 128:(m + 1) * 128],
                            rhs=hts_t[k][:],
                            start=(k == 0), stop=(k == KC - 1))
                    nc.tensor.matmul(
                        pkpe[:], lhsT=wkpe_t[k][:], rhs=hts_t[k][:],
                        start=(k == 0), stop=(k == KC - 1))

                # evict raw latents to bf16 and ship into the gather buffer
                ps_ss = p0ps.tile([1, TOKS], F32, tag="ps_ss", name="ps_ss")
                kvc_sb = []
                for m in range(4):
                    t = p0sb.tile([128, TOKS], BF, tag=f"kvsb{m}", name=f"kvsb{m}")
                    nc.vector.tensor_copy(t[:], pkv[m][:])
                    nc.gpsimd.dma_start(
                        cc2_in[0, m * 128 * TOKS:(m + 1) * 128 * TOKS], t[:])
                    kvc_sb.append(t)
                kpe_sb = p0sb.tile([64, TOKS], BF, tag="kpesb", name="kpesb")
                nc.vector.tensor_copy(kpe_sb[:], pkpe[:])
                nc.gpsimd.dma_start(cc2_in[0, 4 * 128 * TOKS:CC2N], kpe_sb[:])

                # per-token sum of squares over the 512 lora dims (partition
                # reduction via ones-matmul on squared bf16 copies)
                for m in range(4):
                    sq = p0sb.tile([128, TOKS], F32R, tag="sq", name="sq")
                    nc.vector.tensor_mul(sq[:], kvc_sb[m][:], kvc_sb[m][:])
                    nc.tensor.matmul(ps_ss[:], lhsT=ones_col[:], rhs=sq[:],
                                     start=(m == 0), stop=(m == 3))
                rms_kv_row = p0sb.tile([1, TOKS], F32, tag="rmskvr", name="rms_kv_row")
                nc.scalar.activation(rms_kv_row[:], ps_ss[:], AF.Sqrt,
                                     bias=eps_row[:], scale=1.0 / KV_LORA)
                inv_kv_row = p0sb.tile([1, TOKS], F32, tag="invkvr", name="inv_kv_row")
                nc.vector.reciprocal_approx_fast(inv_kv_row[:], rms_kv_row[:])
                nc.gpsimd.dma_start(cc1_in[0, TOKS:2 * TOKS], inv_kv_row[:])

                # ---- q-lora norms (token-major) for own shard ----
                for tb in range(TOKS // 128):
                    tsl = slice(tb * 128, (tb + 1) * 128)
                    ss_q = p0sb.tile([128, 1], F32, tag="ssq")
                    scratch = p0sb.tile([128, 512], F32, tag="scr")
                    for nb3 in range(Q_LORA // 512):
                        ps = p0qps.tile([128, 512], F32, tag="p0q")
                        for k in range(KC):
                            nc.tensor.matmul(
                                ps[:], lhsT=hts_t[k][:, tsl],
                                rhs=wqa_t[k][:, nb3 * 512:(nb3 + 1) * 512],
                                start=(k == 0), stop=(k == KC - 1))
                        ssp = p0sb.tile([128, 1], F32, tag=f"ssp{nb3}", name=f"ssp{nb3}")
                        nc.scalar.activation(scratch[:], ps[:], AF.Square,
                                             accum_out=ssp[:])
                        if nb3 == 0:
                            nc.vector.tensor_copy(ss_q[:], ssp[:])
                        else:
                            nc.vector.tensor_add(ss_q[:], ss_q[:], ssp[:])

                    rms_q = p0sb.tile([128, 1], F32, tag="rmsq")
                    nc.scalar.activation(rms_q[:], ss_q[:], AF.Sqrt,
                                         bias=eps_col[:], scale=1.0 / Q_LORA)
                    inv_q = p0sb.tile([128, 1], F32, tag="invq")
                    nc.vector.reciprocal(inv_q[:], rms_q[:])
                    nc.gpsimd.dma_start(cc_in_slice(cc1_in, tb), inv_q[:])

            nc.gpsimd.collective_compute(
                "AllGather", OP.bypass,
                replica_groups=[list(range(N_CORES))],
                ins=[cc1_in.opt()], outs=[cc1_out.opt()])
            nc.gpsimd.collective_compute(
                "AllGather", OP.bypass,
                replica_groups=[list(range(N_CORES))],
                ins=[cc2_in.opt()], outs=[cc2_out.opt()])

            # persistent activations
            if True:
                acts = top.enter_context(tc.tile_pool(name="acts", bufs=1))
                ivp = top.enter_context(tc.tile_pool(name="ivp", bufs=1))
                qn = [acts.tile([128, TOK], BF, tag=f"qn{h}", name=f"qn{h}") for h in range(HPC)]
                qpe = [acts.tile([64, TOK], BF, tag=f"qpe{h}", name=f"qpe{h}") for h in range(HPC)]
                kn = [acts.tile([128, TOK], BF, tag=f"kn{h}", name=f"kn{h}") for h in range(HPC)]
                kpe = acts.tile([64, TOK], BF, tag="kpe", name="kpe")
                vnat = [acts.tile([128, HPC * D_V], BF, tag=f"v{i}", name=f"v{i}")
                        for i in range(TOK // 128)]
                ivc = [ivp.tile([128, 1], F32, tag=f"ivc{tm}", name=f"ivc{tm}")
                       for tm in range(TOK // 128)]

                # ------------- phase 1: q projection + kv B-projection -----
                with ExitStack() as p1stack:
                    p1h = p1stack.enter_context(tc.tile_pool(name="p1h", bufs=1))
                    p1sb = p1stack.enter_context(tc.tile_pool(name="p1sb", bufs=1))
                    invbc = p1stack.enter_context(tc.tile_pool(name="invbc", bufs=1))
                    kvgp = p1stack.enter_context(tc.tile_pool(name="kvgp", bufs=1))
                    wbp = p1stack.enter_context(tc.tile_pool(name="wbp", bufs=1))

                    inv_kv_bc = invbc.tile([128, TOK], F32, name="inv_kv_bc")
                    bcqp = p1stack.enter_context(tc.tile_pool(name="bcqp", bufs=4))
                    bcq_tiles = [None] * NB

                    # B-projection weights + gathered latents arrive during
                    # the q matmuls
                    wkn_t = []
                    wv_t = []
                    for m in range(4):
                        t = wbp.tile([128, HPC * D_NOPE], BF, tag=f"wkn{m}",
                                     name=f"wkn{m}")
                        nc.gpsimd.dma_start(t[:], WknT[m * 128:(m + 1) * 128, :])
                        wkn_t.append(t)
                        t2 = wbp.tile([128, HPC * D_V], BF, tag=f"wv{m}",
                                      name=f"wv{m}")
                        nc.gpsimd.dma_start(t2[:], WvT[m * 128:(m + 1) * 128, :])
                        wv_t.append(t2)
                    kvg = [kvgp.tile([128, TOK], BF, tag=f"kvg{m}", name=f"kvg{m}")
                           for m in range(4)]
                    kpp = p1stack.enter_context(tc.tile_pool(name="kpp", bufs=2))

                    def evict_q(nbv, psq):
                        tsl = slice(nbv * 512, (nbv + 1) * 512)
                        bq = bcq_tiles[nbv][:]
                        nc.vector.tensor_mul(qn[0][:, tsl], psq[0][:], bq)
                        nc.vector.tensor_mul(qn[1][:, tsl], psq[1][:], bq)
                        # rope q_pe stack [E0 E1 O0 O1] (scaled by inv_q)
                        tq = p1sb.tile([128, 512], F32, tag="tq", name="tq")
                        nc.vector.tensor_mul(tq[:], psq[2][:], bq)
                        m1a = p1sb.tile([64, 512], F32, tag="m1a", name="m1a")
                        m1b = p1sb.tile([64, 512], F32, tag="m1b", name="m1b")
                        # tq rows [E0 E1 O0 O1]; csq1=[C C S S], csq2=[S S C C]
                        nc.vector.tensor_mul(m1a[:], tq[0:64, :], csq1_t[0:64, tsl])
                        nc.vector.tensor_mul(m1b[:], tq[64:128, :], csq1_t[64:128, tsl])
                        nc.vector.tensor_sub(qpe[0][0:32, tsl], m1a[0:32, :], m1b[0:32, :])
                        nc.vector.tensor_sub(qpe[1][0:32, tsl], m1a[32:64, :], m1b[32:64, :])
                        m2a = p1sb.tile([64, 512], F32, tag="m1a", name="m2a")
                        m2b = p1sb.tile([64, 512], F32, tag="m1b", name="m2b")
                        nc.vector.tensor_mul(m2a[:], tq[0:64, :], csq2_t[0:64, tsl])
                        nc.vector.tensor_mul(m2b[:], tq[64:128, :], csq2_t[64:128, tsl])
                        nc.vector.tensor_add(qpe[0][32:64, tsl], m2a[0:32, :], m2b[0:32, :])
                        nc.vector.tensor_add(qpe[1][32:64, tsl], m2a[32:64, :], m2b[32:64, :])

                    with ExitStack() as pqs:
                        pqps = pqs.enter_context(
                            tc.tile_pool(name="pqps", bufs=2, space="PSUM"))
                        prev = None
                        for nb in range(NB):         # 8 blocks of 512 tokens
                            tsl = slice(nb * 512, (nb + 1) * 512)
                            ht_c = []
                            for k in range(KC):
                                t = p1h.tile([128, 512], BF, tag=f"htc{k}",
                                             name=f"htc{k}")
                                eng = nc.sync if k % 2 == 0 else nc.scalar
                                eng.dma_start(t[:], hT[k * 128:(k + 1) * 128, tsl])
                                ht_c.append(t)
                            if 1 <= nb <= 4:
                                # inv-rms broadcasts (DRAM->SBUF, partition
                                # bcast) trickled in behind cc1
                                for j in (2 * (nb - 1), 2 * nb - 1):
                                    sl = slice(j * 512, (j + 1) * 512)
                                    t = bcqp.tile([128, 512], F32, tag="bcq",
                                                  name=f"bcq{j}")
                                    nc.scalar.dma_start(
                                        t[:],
                                        cc1_out[j:j + 1, 0:TOKS]
                                        .to_broadcast((128, TOKS)))
                                    bcq_tiles[j] = t
                                    nc.scalar.dma_start(
                                        inv_kv_bc[:, sl],
                                        cc1_out[j:j + 1, TOKS:2 * TOKS]
                                        .to_broadcast((128, TOKS)))
                            ps_q = [pqps.tile([128, 512], F32, tag=f"pq{mb}",
                                              name=f"pq{mb}") for mb in range(3)]
                            for k in range(KC):
                                for mb in range(3):
                                    nc.tensor.matmul(
                                        ps_q[mb][:],
                                        lhsT=wq_t[k][:, mb * 128:(mb + 1) * 128],
                                        rhs=ht_c[k][:],
                                        start=(k == 0), stop=(k == KC - 1))
                            # evictions lag one block so the first broadcast
                            # (which waits on the gather) never stalls matmuls
                            if prev is not None:
                                evict_q(*prev)
                            prev = (nb, ps_q)
                        evict_q(*prev)

                    # gathered latents: emitted after all hT streaming so the
                    # cc2 dependency never blocks the sync queue's prefetches
                    kpe_raws = []
                    for r in range(NB):
                        for m in range(4):
                            nc.sync.dma_start(
                                kvg[m][:, r * 512:(r + 1) * 512],
                                cc2_out[r, m * 128 * TOKS:(m + 1) * 128 * TOKS])
                        kr = kpp.tile([64, 512], BF, tag="kpe_raw",
                                      name=f"kpe_raw{r}")
                        nc.gpsimd.dma_start(
                            kr[:], cc2_out[r, 4 * 128 * TOKS:CC2N])
                        kpe_raws.append(kr)

                    # ---- kv B-projection over all gathered tokens ----
                    with ExitStack() as pbs:
                        pbps = pbs.enter_context(
                            tc.tile_pool(name="pbps", bufs=2, space="PSUM"))
                        for r in range(NB):
                            rsl = slice(r * 512, (r + 1) * 512)
                            for h in range(HPC):
                                pkn = pbps.tile([128, 512], F32, tag="pkn",
                                                name="pkn")
                                for m in range(4):
                                    nc.tensor.matmul(
                                        pkn[:],
                                        lhsT=wkn_t[m][:, h * D_NOPE:
                                                      (h + 1) * D_NOPE],
                                        rhs=kvg[m][:, rsl],
                                        start=(m == 0), stop=(m == 3))
                                nc.vector.tensor_mul(kn[h][:, rsl], pkn[:],
                                                     inv_kv_bc[:, rsl])
                            # v for the 4 128-token chunks of this shard
                            for half in range(2):
                                pv = pbps.tile([128, 512], F32, tag="pv",
                                               name="pv")
                                for sub in range(2):
                                    tm = r * 4 + half * 2 + sub
                                    for m in range(4):
                                        nc.tensor.matmul(
                                            pv[:, sub * 256:(sub + 1) * 256],
                                            lhsT=kvg[m][:, tm * 128:
                                                        (tm + 1) * 128],
                                            rhs=wv_t[m][:],
                                            start=(m == 0 and sub == 0),
                                            stop=(m == 3 and sub == 1))
                                for sub in range(2):
                                    tm = r * 4 + half * 2 + sub
                                    nc.scalar.dma_start(
                                        ivc[tm][:],
                                        cc1_out[tm // 4, TOKS + (tm % 4) * 128:
                                                TOKS + (tm % 4) * 128 + 128])
                                    nc.scalar.mul(
                                        vnat[tm][:],
                                        pv[:, sub * 256:(sub + 1) * 256],
                                        ivc[tm][:])
                            # rope k_pe for this shard [E O]; csk1=[C S], csk2=[S C]
                            mka = p1sb.tile([32, 512], F32, tag="mka", name="mka")
                            mkb = p1sb.tile([32, 512], F32, tag="mkb", name="mkb")
                            nc.vector.tensor_mul(mka[:], kpe_raws[r][0:32, :],
                                                 csk1_t[0:32, rsl])
                            nc.vector.tensor_mul(mkb[:], kpe_raws[r][32:64, :],
                                                 csk1_t[32:64, rsl])
                            nc.vector.tensor_sub(kpe[0:32, rsl], mka[:], mkb[:])
                            mkc = p1sb.tile([32, 512], F32, tag="mka", name="mkc")
                            mkd = p1sb.tile([32, 512], F32, tag="mkb", name="mkd")
                            nc.vector.tensor_mul(mkc[:], kpe_raws[r][0:32, :],
                                                 csk2_t[0:32, rsl])
                            nc.vector.tensor_mul(mkd[:], kpe_raws[r][32:64, :],
                                                 csk2_t[32:64, rsl])
                            nc.vector.tensor_add(kpe[32:64, rsl], mkc[:], mkd[:])

                # ------------- phase 2+3: attention + wo -------------------
                if True:
                    wop = top.enter_context(tc.tile_pool(name="wop", bufs=1))
                    sps = top.enter_context(
                        tc.tile_pool(name="sps", bufs=1, space="PSUM"))
                    ops = top.enter_context(
                        tc.tile_pool(name="ops", bufs=1, space="PSUM"))
                    dps = top.enter_context(
                        tc.tile_pool(name="dps", bufs=1, space="PSUM"))
                    wps = top.enter_context(
                        tc.tile_pool(name="wps", bufs=1, space="PSUM"))
                    bnc = top.enter_context(
                        tc.tile_pool(name="bnc", bufs=4, space="DRAM"))
                    esb = top.enter_context(tc.tile_pool(name="esb", bufs=2))
                    asb = top.enter_context(tc.tile_pool(name="asb", bufs=2))
                    nsb = top.enter_context(tc.tile_pool(name="nsb", bufs=2))
                    otp = top.enter_context(tc.tile_pool(name="otp", bufs=2))
                    osb = top.enter_context(tc.tile_pool(name="osb", bufs=3))

                    wo_t = []
                    for i in range(2):
                        t = wop.tile([128, H], BF, tag=f"wot{i}", name=f"wot{i}")
                        nc.gpsimd.dma_start(t[:], woR[i * 128:(i + 1) * 128, :])
                        wo_t.append(t)

                    # deferred tensor-engine work, drained into slots of
                    # subsequent attention streams
                    pend = []

                    def drain_one():
                        if pend:
                            pend.pop(0)()

                    def make_den_head(ps_o, acc_d, outT_h, osl):
                        def emit():
                            ps_d = dps.tile([1, 512], F32, tag="ps_d", name="ps_d")
                            nc.tensor.matmul(
                                ps_d[:], lhsT=ones_col[:], rhs=acc_d[:, 0:512],
                                start=True, stop=False)
                            nc.tensor.matmul(
                                ps_d[:], lhsT=ones_col[:],
                                rhs=acc_d[:, 512:1024],
                                start=False, stop=True)
                            rec = nsb.tile([1, 512], F32, tag="rec", name="rec")
                            nc.vector.reciprocal_approx_fast(rec[:], ps_d[:])
                            rec_d = bnc.tile([1, 512], F32, tag="rec_d",
                                             name="rec_d")
                            nc.sync.dma_start(rec_d[:], rec[:])
                            pend.insert(1, make_norm_tail(ps_o, rec_d, outT_h, osl))
                        return emit

                    def make_norm_tail(ps_o, rec_d, outT_h, osl):
                        def emit():
                            # broadcast 1/den across partitions via DRAM bounce
                            bc_sb = nsb.tile([128, 512], F32, tag="bc_sb",
                                             name="bc_sb")
                            nc.sync.dma_start(
                                bc_sb[:], rec_d[0:1, :].to_broadcast((128, 512)))
                            nc.vector.tensor_mul(outT_h[:, osl], ps_o[:], bc_sb[:])
                        return emit

                    def make_wo_group(outT, trow, osl128, hn, par):
                        def emit():
                            ps_w = wps.tile([128, 512], F32, tag="ps_w", name="ps_w")
                            for h in range(HPC):
                                nc.tensor.matmul(
                                    ps_w[:],
                                    lhsT=outT[h][:, osl128],
                                    rhs=wo_t[h][:, hn * 512:(hn + 1) * 512],
                                    start=(h == 0), stop=(h == HPC - 1))
                            o_sb = osb.tile([128, 512], BF, tag="o_sb", name="o_sb")
                            if par == 0:
                                nc.vector.tensor_copy(o_sb[:], ps_w[:])
                            else:
                                nc.scalar.copy(o_sb[:], ps_w[:])
                            nc.sync.dma_start(
                                out[trow:trow + 128, hn * 512:(hn + 1) * 512],
                                o_sb[:])
                        return emit

                    for b in range(B):
                        outT = [otp.tile([128, S], BF, tag=f"outT{h}", name=f"outT{h}")
                                for h in range(HPC)]
                        for qb in range(S // 512):
                            qsl = slice(b * S + qb * 512, b * S + qb * 512 + 512)
                            osl = slice(qb * 512, qb * 512 + 512)
                            ps_o = [ops.tile([128, 512], F32, tag=f"ps_o{h}",
                                             name=f"ps_o{h}") for h in range(HPC)]
                            acc_d = [asb.tile([128, 1024], F32R, tag=f"acc{h}",
                                              name=f"acc{h}") for h in range(HPC)]
                            # two k-chunks share one [128,1024] score tile so
                            # exp and the denominator adds run 1024-wide
                            pss = [None, None]
                            pex = [[None, None], [None, None]]
                            for kc in range(S // 128):
                                ksl = slice(b * S + kc * 128,
                                            b * S + kc * 128 + 128)
                                p = kc // 2
                                hf = (kc % 2) * 512
                                for h in range(HPC):
                                    if kc % 2 == 0:
                                        pss[h] = sps.tile([128, 1024], F32,
                                                          tag=f"ps_s{h}",
                                                          name=f"ps_s{h}")
                                        pex[p % 2][h] = esb.tile(
                                            [128, 1024], BF, tag=f"exp{h}",
                                            name=f"exp{h}")
                                    nc.tensor.matmul(pss[h][:, hf:hf + 512],
                                                     lhsT=kn[h][:, ksl],
                                                     rhs=qn[h][:, qsl],
                                                     start=True, stop=False)
                                    nc.tensor.matmul(
                                        pss[h][:, hf:hf + 512], lhsT=kpe[:, ksl],
                                        rhs=qpe[h][:, qsl],
                                        start=False, stop=True)
                                    if kc % 2 == 1:
                                        pt = pex[p % 2][h]
                                        nc.scalar.activation(pt[:], pss[h][:],
                                                             AF.Exp, scale=SCALE)
                                        if kc == 1:
                                            nc.vector.tensor_copy(acc_d[h][:],
                                                                  pt[:])
                                        else:
                                            nc.vector.tensor_add(acc_d[h][:],
                                                                 acc_d[h][:],
                                                                 pt[:])
                                if kc % 2 == 1 and p >= 1:
                                    for h in range(HPC):
                                        pt = pex[(p - 1) % 2][h]
                                        for sub in range(2):
                                            kcp = 2 * (p - 1) + sub
                                            tm = (b * S) // 128 + kcp
                                            nc.tensor.matmul(
                                                ps_o[h][:],
                                                lhsT=vnat[tm][:, h * D_V:
                                                             (h + 1) * D_V],
                                                rhs=pt[:, sub * 512:
                                                       sub * 512 + 512],
                                                start=(kcp == 0), stop=False)
                                if kc % 2 == 1:
                                    drain_one()
                                    drain_one()
                                    if kc >= 9:
                                        drain_one()
                            for h in range(HPC):
                                pt = pex[1][h]
                                for sub in range(2):
                                    kcp = 14 + sub
                                    tm = (b * S) // 128 + kcp
                                    nc.tensor.matmul(
                                        ps_o[h][:],
                                        lhsT=vnat[tm][:, h * D_V:(h + 1) * D_V],
                                        rhs=pt[:, sub * 512:sub * 512 + 512],
                                        start=False, stop=(kcp == 15))
                            for h in range(HPC):
                                pend.append(make_den_head(ps_o[h], acc_d[h],
                                                          outT[h], osl))
                            # queue wo for this (b, qb) once both heads are in
                            for j in range(4):
                                tmb = qb * 4 + j
                                osl128 = slice(tmb * 128, tmb * 128 + 128)
                                trow = b * S + tmb * 128
                                for hn in range(H // 512):
                                    pend.append(make_wo_group(
                                        outT, trow, osl128, hn, (j + hn) % 2))

                    while pend:
                        drain_one()

    nc.compile()
    return nc


def cc_in_slice(cc1_in, tb):
    return cc1_in[0, tb * 128:(tb + 1) * 128]


_PROGRAM = None


def _get_program():
    global _PROGRAM
    if _PROGRAM is None:
        _PROGRAM = _build_program()
    return _PROGRAM


def kernel(hidden_states, wq_a, q_norm_w, wq_b, wkv_a, kv_norm_w, wkv_b, wo):
    nc = _get_program()
    in_maps = _host_prep(hidden_states, wq_a, q_norm_w, wq_b,
                         wkv_a, kv_norm_w, wkv_b, wo)
    res = run_bass_kernel_spmd(nc, in_maps, list(range(N_CORES)))
    total = np.zeros((TOK, H), dtype=np.float32)
    for r in res.results:
        total += np.asarray(r["out"], dtype=np.float32)
    return total.reshape(B, S, H)


# revision 43
# speedup vs baseline: 1.0535x; 1.0535x over previous
"""MLA (multi-head latent attention) forward, sharded over 8 TRN2 NeuronCores.

Tensor-parallel over heads (2 heads/core).  The q path folds rmsnorm into the
B-projection and fuses A@B into per-head weights (rmsnorm's per-token scale
commutes through the matmul).  The kv path instead exploits the low-rank
structure: each core computes the 512-dim latent kv_c (and rope'd k_pe input)
for its OWN 512-token shard only, the shards are AllGather'd in bf16, and
each core then applies its heads' 512->256 B-projections — much cheaper than
fusing through the 2048-dim hidden.  A second, tiny AllGather shares the
per-token inverse-rms scalars.

Phase 2 runs attention per (batch, 512-query block) with both heads' streams
interleaved so every engine has two independent chains: scores^T in PSUM,
exp on the scalar engine, denominator via a ones-matmul in fp32r (single
pass), fast-approx reciprocal, softmax normalization + wo matmuls deferred
into drain slots of later streams.  wo output is written bf16; the host sums
the 8 partial outputs (the "all-reduce after wo" of the sharding hint).

Matmuls run in bf16 with fp32 PSUM accumulation; softmax and statistics stay
fp32.  RoPE features are permuted to a split even/odd layout on both q and k
(host permutes the projection weight rows identically, dot products unchanged).
"""
import sys
from contextlib import ExitStack

sys.path.insert(0, "/opt/trn_rl_repo")

import numpy as np
import ml_dtypes

import concourse.mybir as mybir
from concourse import bacc
from concourse.tile import TileContext
from concourse.bass_utils import run_bass_kernel_spmd

BF16 = ml_dtypes.bfloat16
F32 = mybir.dt.float32
F32R = mybir.dt.float32r
BF = mybir.dt.bfloat16

B, S, H = 2, 2048, 2048
NH = 16
Q_LORA, KV_LORA = 1536, 512
D_NOPE, D_ROPE, D_V = 128, 64, 128
D_QK = D_NOPE + D_ROPE
SCALE = 1.0 / float(np.sqrt(D_QK))
EPS = 1e-6

N_CORES = 8
HPC = NH // N_CORES          # heads per core = 2
TOK = B * S                  # 4096
TOKS = TOK // N_CORES        # 512-token shard per core
KC = H // 128                # 16 contraction chunks over hidden features
NB = TOK // 512              # 8 token blocks of 512
NQ = 384                     # fused q features per core (2*128 nope + 128 pe)
CC2N = 4 * 128 * TOKS + 64 * TOKS   # bf16 elems per core in the kv gather


def _host_tables():
    inv = 1.0 / (10000.0 ** (np.arange(0, D_ROPE, 2, dtype=np.float32) / D_ROPE))
    t = np.arange(S, dtype=np.float32)
    f = np.outer(t, inv)                       # (S, 32)
    cos = np.tile(np.cos(f).T, (1, B))         # (32, TOK), tokens b-major
    sin = np.tile(np.sin(f).T, (1, B))
    csq1 = np.concatenate([cos, cos, sin, sin], axis=0)   # (128, TOK)
    csq2 = np.concatenate([sin, sin, cos, cos], axis=0)
    csk1 = np.concatenate([cos, sin], axis=0)             # (64, TOK)
    csk2 = np.concatenate([sin, cos], axis=0)
    return [np.ascontiguousarray(x).astype(BF16) for x in (csq1, csq2, csk1, csk2)]


def _host_prep(hidden_states, wq_a, q_norm_w, wq_b, wkv_a, kv_norm_w, wkv_b, wo):
    hid = np.ascontiguousarray(np.asarray(hidden_states, dtype=np.float32).reshape(TOK, H))
    hT_bf = np.ascontiguousarray(hid.T).astype(BF16)             # (H, TOK)

    wq_b_f = (np.asarray(wq_b) * np.asarray(q_norm_w)[None, :]).astype(np.float32)
    wkv_b_f = (np.asarray(wkv_b) * np.asarray(kv_norm_w)[None, :]).astype(np.float32)

    Wq = wq_b_f @ np.asarray(wq_a)                 # (NH*192, H)
    wkpe = np.asarray(wkv_a)[KV_LORA:]             # (64, H)

    ev = np.arange(0, D_ROPE, 2)
    od = np.arange(1, D_ROPE, 2)
    csq1, csq2, csk1, csk2 = _host_tables()

    wqaT = np.ascontiguousarray(np.asarray(wq_a).T).astype(BF16)
    wkvaT = np.ascontiguousarray(np.asarray(wkv_a)[:KV_LORA].T).astype(BF16)
    wkpeT = np.ascontiguousarray(
        np.concatenate([wkpe[ev], wkpe[od]], axis=0).T).astype(BF16)   # (H, 64)

    in_maps = []
    for c in range(N_CORES):
        h0, h1 = 2 * c, 2 * c + 1
        qh = [Wq[h * D_QK:(h + 1) * D_QK] for h in (h0, h1)]
        qpe0, qpe1 = qh[0][D_NOPE:], qh[1][D_NOPE:]
        W_q_all = np.concatenate([
            qh[0][:D_NOPE], qh[1][:D_NOPE],
            qpe0[ev], qpe1[ev], qpe0[od], qpe1[od],
        ], axis=0)                                               # (384, H)
        WqT = np.ascontiguousarray(W_q_all.T).astype(BF16)       # (H, 384)

        bh = [wkv_b_f[h * (D_NOPE + D_V):(h + 1) * (D_NOPE + D_V)] for h in (h0, h1)]
        WknT = np.ascontiguousarray(np.concatenate(
            [bh[0][:D_NOPE], bh[1][:D_NOPE]], axis=0).T).astype(BF16)   # (512, 256)
        WvT = np.ascontiguousarray(np.concatenate(
            [bh[0][D_NOPE:], bh[1][D_NOPE:]], axis=0).T).astype(BF16)   # (512, 256)

        wo_h = np.asarray(wo)[:, c * HPC * D_V:(c + 1) * HPC * D_V]   # (H, 256)
        woR = np.ascontiguousarray(wo_h.T).astype(BF16)          # (256, H)

        in_maps.append({
            "hT": hT_bf,
            "hTs": np.ascontiguousarray(hT_bf[:, c * TOKS:(c + 1) * TOKS]),
            "wqaT": wqaT,
            "wkvaT": wkvaT,
            "wkpeT": wkpeT,
            "WqT": WqT,
            "WknT": WknT,
            "WvT": WvT,
            "woR": woR,
            "csq1": csq1, "csq2": csq2, "csk1": csk1, "csk2": csk2,
        })
    return in_maps


def _build_program():
    nc = bacc.Bacc()

    hT = nc.dram_tensor("hT", [H, TOK], BF, kind="ExternalInput")
    hTs = nc.dram_tensor("hTs", [H, TOKS], BF, kind="ExternalInput")
    wqaT = nc.dram_tensor("wqaT", [H, Q_LORA], BF, kind="ExternalInput")
    wkvaT = nc.dram_tensor("wkvaT", [H, KV_LORA], BF, kind="ExternalInput")
    wkpeT = nc.dram_tensor("wkpeT", [H, 64], BF, kind="ExternalInput")
    WqT = nc.dram_tensor("WqT", [H, NQ], BF, kind="ExternalInput")
    WknT = nc.dram_tensor("WknT", [KV_LORA, HPC * D_NOPE], BF, kind="ExternalInput")
    WvT = nc.dram_tensor("WvT", [KV_LORA, HPC * D_V], BF, kind="ExternalInput")
    woR = nc.dram_tensor("woR", [HPC * D_V, H], BF, kind="ExternalInput")
    csq1d = nc.dram_tensor("csq1", [128, TOK], BF, kind="ExternalInput")
    csq2d = nc.dram_tensor("csq2", [128, TOK], BF, kind="ExternalInput")
    csk1d = nc.dram_tensor("csk1", [64, TOK], BF, kind="ExternalInput")
    csk2d = nc.dram_tensor("csk2", [64, TOK], BF, kind="ExternalInput")
    out = nc.dram_tensor("out", [TOK, H], BF, kind="ExternalOutput")

    AF = mybir.ActivationFunctionType
    OP = mybir.AluOpType

    with TileContext(nc) as tc, ExitStack() as top:
        if True:
            onesp = top.enter_context(tc.tile_pool(name="onesp", bufs=1))
            ccp = top.enter_context(tc.tile_pool(name="ccp", bufs=1, space="DRAM"))
            wqp = top.enter_context(tc.tile_pool(name="wqp", bufs=1))
            csp = top.enter_context(tc.tile_pool(name="csp", bufs=1))
            p1h = top.enter_context(tc.tile_pool(name="p1h", bufs=1))
            ht_tiles = {}

            def load_ht(nb, engs):
                tsl = slice(nb * 512, (nb + 1) * 512)
                for k in range(KC):
                    t = p1h.tile([128, 512], BF, tag=f"htc{k}", name=f"htc{k}")
                    engs[k % len(engs)].dma_start(
                        t[:], hT[k * 128:(k + 1) * 128, tsl])
                    ht_tiles[(nb, k)] = t

            ones_col_f = onesp.tile([128, 1], F32)
            ones_col = onesp.tile([128, 1], F32R)
            eps_col = onesp.tile([128, 1], F32)
            eps_row = onesp.tile([1, 1], F32)
            nc.vector.memset(ones_col_f[:], 1.0)
            nc.vector.tensor_copy(ones_col[:], ones_col_f[:])
            nc.vector.memset(eps_col[:], EPS)
            nc.vector.memset(eps_row[:], EPS)
            cc1_in = ccp.tile([1, 2 * TOKS], F32)
            cc1_out = ccp.tile([N_CORES, 2 * TOKS], F32, addr_space="Shared")
            cc2_in = ccp.tile([1, CC2N], BF)
            cc2_out = ccp.tile([N_CORES, CC2N], BF, addr_space="Shared")

            # ---------------- phase 0: latent kv + rms scalars -------------
            with ExitStack() as p0stack:
                p0w = p0stack.enter_context(tc.tile_pool(name="p0w", bufs=1))
                p0ps = p0stack.enter_context(
                    tc.tile_pool(name="p0ps", bufs=1, space="PSUM"))
                p0qps = p0stack.enter_context(
                    tc.tile_pool(name="p0qps", bufs=2, space="PSUM"))
                p0sb = p0stack.enter_context(tc.tile_pool(name="p0sb", bufs=2))

                # spread startup loads over all three DMA-issuing queues
                hts_t = []
                for k in range(KC):
                    t = p0w.tile([128, TOKS], BF, tag=f"hts{k}", name=f"hts{k}")
                    eng = nc.sync if k % 2 == 0 else nc.scalar
                    eng.dma_start(t[:], hTs[k * 128:(k + 1) * 128, :])
                    hts_t.append(t)
                wkva_t = []
                for k in range(KC):
                    t = p0w.tile([128, KV_LORA], BF, tag=f"wkva{k}", name=f"wkva{k}")
                    nc.gpsimd.dma_start(t[:], wkvaT[k * 128:(k + 1) * 128, :])
                    wkva_t.append(t)
                wkpe_t = []
                for k in range(KC):
                    t = p0w.tile([128, 64], BF, tag=f"wkpe{k}", name=f"wkpe{k}")
                    nc.gpsimd.dma_start(t[:], wkpeT[k * 128:(k + 1) * 128, :])
                    wkpe_t.append(t)
                # wqa in three 512-column blocks so q-norm matmuls can start
                # before the whole 6.3MB lands
                wqa_t = []
                for k in range(KC):
                    t = p0w.tile([128, Q_LORA], BF, tag=f"wqa{k}", name=f"wqa{k}")
                    wqa_t.append(t)
                for nb3 in range(Q_LORA // 512):
                    for k in range(KC):
                        eng = (nc.sync, nc.scalar, nc.gpsimd)[k % 3]
                        eng.dma_start(
                            wqa_t[k][:, nb3 * 512:(nb3 + 1) * 512],
                            wqaT[k * 128:(k + 1) * 128, nb3 * 512:(nb3 + 1) * 512])

                # phase-1 weight prefetch behind the p0 loads
                wq_t = []
                for k in range(KC):
                    t = wqp.tile([128, NQ], BF, tag=f"wq{k}", name=f"wq{k}")
                    eng = (nc.sync, nc.scalar, nc.gpsimd)[k % 3]
                    eng.dma_start(t[:], WqT[k * 128:(k + 1) * 128, :])
                    wq_t.append(t)
                csq1_t = csp.tile([128, TOK], BF, tag="csq1", name="csq1")
                csq2_t = csp.tile([128, TOK], BF, tag="csq2", name="csq2")
                nc.sync.dma_start(csq1_t[:], csq1d[:])
                nc.scalar.dma_start(csq2_t[:], csq2d[:])
                csk1_t = csp.tile([64, TOK], BF, tag="csk1", name="csk1")
                csk2_t = csp.tile([64, TOK], BF, tag="csk2", name="csk2")
                nc.sync.dma_start(csk1_t[:], csk1d[:])
                nc.scalar.dma_start(csk2_t[:], csk2d[:])

                # ---- latent kv (lora-major) for own shard ----
                pkv = [p0ps.tile([128, TOKS], F32, tag=f"pkv{m}", name=f"pkv{m}")
                       for m in range(4)]
                pkpe = p0ps.tile([64, TOKS], F32, tag="pkpe", name="pkpe")
                for k in range(KC):
                    for m in range(4):
                        nc.tensor.matmul(
                            pkv[m][:],
                            lhsT=wkva_t[k][:, m * 128:(m + 1) * 128],
                            rhs=hts_t[k][:],
                            start=(k == 0), stop=(k == KC - 1))
                    nc.tensor.matmul(
                        pkpe[:], lhsT=wkpe_t[k][:], rhs=hts_t[k][:],
                        start=(k == 0), stop=(k == KC - 1))

                # evict raw latents to bf16 and ship into the gather buffer
                ps_ss = p0ps.tile([1, TOKS], F32, tag="ps_ss", name="ps_ss")
                kvc_sb = []
                for m in range(4):
                    t = p0sb.tile([128, TOKS], BF, tag=f"kvsb{m}", name=f"kvsb{m}")
                    nc.vector.tensor_copy(t[:], pkv[m][:])
                    nc.gpsimd.dma_start(
                        cc2_in[0, m * 128 * TOKS:(m + 1) * 128 * TOKS], t[:])
                    kvc_sb.append(t)
                kpe_sb = p0sb.tile([64, TOKS], BF, tag="kpesb", name="kpesb")
                nc.vector.tensor_copy(kpe_sb[:], pkpe[:])
                nc.gpsimd.dma_start(cc2_in[0, 4 * 128 * TOKS:CC2N], kpe_sb[:])

                # per-token sum of squares over the 512 lora dims (partition
                # reduction via ones-matmul on squared bf16 copies)
                for m in range(4):
                    sq = p0sb.tile([128, TOKS], F32R, tag="sq", name="sq")
                    nc.vector.tensor_mul(sq[:], kvc_sb[m][:], kvc_sb[m][:])
                    nc.tensor.matmul(ps_ss[:], lhsT=ones_col[:], rhs=sq[:],
                                     start=(m == 0), stop=(m == 3))
                rms_kv_row = p0sb.tile([1, TOKS], F32, tag="rmskvr", name="rms_kv_row")
                nc.scalar.activation(rms_kv_row[:], ps_ss[:], AF.Sqrt,
                                     bias=eps_row[:], scale=1.0 / KV_LORA)
                inv_kv_row = p0sb.tile([1, TOKS], F32, tag="invkvr", name="inv_kv_row")
                nc.vector.reciprocal_approx_fast(inv_kv_row[:], rms_kv_row[:])
                nc.gpsimd.dma_start(cc1_in[0, TOKS:2 * TOKS], inv_kv_row[:])

                # ---- q-lora norms (token-major) for own shard ----
                for tb in range(TOKS // 128):
                    tsl = slice(tb * 128, (tb + 1) * 128)
                    ss_q = p0sb.tile([128, 1], F32, tag="ssq")
                    scratch = p0sb.tile([128, 512], F32, tag="scr")
                    for nb3 in range(Q_LORA // 512):
                        ps = p0qps.tile([128, 512], F32, tag="p0q")
                        for k in range(KC):
                            nc.tensor.matmul(
                                ps[:], lhsT=hts_t[k][:, tsl],
                                rhs=wqa_t[k][:, nb3 * 512:(nb3 + 1) * 512],
                                start=(k == 0), stop=(k == KC - 1))
                        ssp = p0sb.tile([128, 1], F32, tag=f"ssp{nb3}", name=f"ssp{nb3}")
                        nc.scalar.activation(scratch[:], ps[:], AF.Square,
                                             accum_out=ssp[:])
                        if nb3 == 0:
                            nc.vector.tensor_copy(ss_q[:], ssp[:])
                        else:
                            nc.vector.tensor_add(ss_q[:], ss_q[:], ssp[:])

                    rms_q = p0sb.tile([128, 1], F32, tag="rmsq")
                    nc.scalar.activation(rms_q[:], ss_q[:], AF.Sqrt,
                                         bias=eps_col[:], scale=1.0 / Q_LORA)
                    inv_q = p0sb.tile([128, 1], F32, tag="invq")
                    nc.vector.reciprocal(inv_q[:], rms_q[:])
                    nc.gpsimd.dma_start(cc_in_slice(cc1_in, tb), inv_q[:])

            load_ht(0, [nc.sync, nc.scalar])
            load_ht(1, [nc.sync, nc.scalar])

            nc.gpsimd.collective_compute(
                "AllGather", OP.bypass,
                replica_groups=[list(range(N_CORES))],
                ins=[cc1_in.opt()], outs=[cc1_out.opt()])
            nc.gpsimd.collective_compute(
                "AllGather", OP.bypass,
                replica_groups=[list(range(N_CORES))],
                ins=[cc2_in.opt()], outs=[cc2_out.opt()])

            # persistent activations
            if True:
                acts = top.enter_context(tc.tile_pool(name="acts", bufs=1))
                ivp = top.enter_context(tc.tile_pool(name="ivp", bufs=1))
                qn = [acts.tile([128, TOK], BF, tag=f"qn{h}", name=f"qn{h}") for h in range(HPC)]
                qpe = [acts.tile([64, TOK], BF, tag=f"qpe{h}", name=f"qpe{h}") for h in range(HPC)]
                kn = [acts.tile([128, TOK], BF, tag=f"kn{h}", name=f"kn{h}") for h in range(HPC)]
                kpe = acts.tile([64, TOK], BF, tag="kpe", name="kpe")
                vnat = [acts.tile([128, HPC * D_V], BF, tag=f"v{i}", name=f"v{i}")
                        for i in range(TOK // 128)]
                ivc = [ivp.tile([128, 1], F32, tag=f"ivc{tm}", name=f"ivc{tm}")
                       for tm in range(TOK // 128)]

                # ------------- phase 1: q projection + kv B-projection -----
                with ExitStack() as p1stack:
                    p1sb = p1stack.enter_context(tc.tile_pool(name="p1sb", bufs=1))
                    invbc = p1stack.enter_context(tc.tile_pool(name="invbc", bufs=1))
                    kvgp = p1stack.enter_context(tc.tile_pool(name="kvgp", bufs=1))
                    wbp = p1stack.enter_context(tc.tile_pool(name="wbp", bufs=1))

                    inv_kv_bc = invbc.tile([128, TOK], F32, name="inv_kv_bc")
                    bcqp = p1stack.enter_context(tc.tile_pool(name="bcqp", bufs=4))
                    bcq_tiles = [None] * NB

                    # B-projection weights + gathered latents arrive during
                    # the q matmuls
                    wkn_t = []
                    wv_t = []
                    for m in range(4):
                        t = wbp.tile([128, HPC * D_NOPE], BF, tag=f"wkn{m}",
                                     name=f"wkn{m}")
                        nc.gpsimd.dma_start(t[:], WknT[m * 128:(m + 1) * 128, :])
                        wkn_t.append(t)
                        t2 = wbp.tile([128, HPC * D_V], BF, tag=f"wv{m}",
                                      name=f"wv{m}")
                        nc.gpsimd.dma_start(t2[:], WvT[m * 128:(m + 1) * 128, :])
                        wv_t.append(t2)
                    kvg = [kvgp.tile([128, TOK], BF, tag=f"kvg{m}", name=f"kvg{m}")
                           for m in range(4)]
                    kpp = p1stack.enter_context(tc.tile_pool(name="kpp", bufs=2))

                    def evict_q(nbv, psq):
                        tsl = slice(nbv * 512, (nbv + 1) * 512)
                        bq = bcq_tiles[nbv][:]
                        nc.vector.tensor_mul(qn[0][:, tsl], psq[0][:], bq)
                        nc.vector.tensor_mul(qn[1][:, tsl], psq[1][:], bq)
                        # rope q_pe stack [E0 E1 O0 O1] (scaled by inv_q)
                        tq = p1sb.tile([128, 512], F32, tag="tq", name="tq")
                        nc.vector.tensor_mul(tq[:], psq[2][:], bq)
                        m1a = p1sb.tile([64, 512], F32, tag="m1a", name="m1a")
                        m1b = p1sb.tile([64, 512], F32, tag="m1b", name="m1b")
                        # tq rows [E0 E1 O0 O1]; csq1=[C C S S], csq2=[S S C C]
                        nc.vector.tensor_mul(m1a[:], tq[0:64, :], csq1_t[0:64, tsl])
                        nc.vector.tensor_mul(m1b[:], tq[64:128, :], csq1_t[64:128, tsl])
                        nc.vector.tensor_sub(qpe[0][0:32, tsl], m1a[0:32, :], m1b[0:32, :])
                        nc.vector.tensor_sub(qpe[1][0:32, tsl], m1a[32:64, :], m1b[32:64, :])
                        m2a = p1sb.tile([64, 512], F32, tag="m1a", name="m2a")
                        m2b = p1sb.tile([64, 512], F32, tag="m1b", name="m2b")
                        nc.vector.tensor_mul(m2a[:], tq[0:64, :], csq2_t[0:64, tsl])
                        nc.vector.tensor_mul(m2b[:], tq[64:128, :], csq2_t[64:128, tsl])
                        nc.vector.tensor_add(qpe[0][32:64, tsl], m2a[0:32, :], m2b[0:32, :])
                        nc.vector.tensor_add(qpe[1][32:64, tsl], m2a[32:64, :], m2b[32:64, :])

                    with ExitStack() as pqs:
                        pqps = pqs.enter_context(
                            tc.tile_pool(name="pqps", bufs=2, space="PSUM"))
                        prev = None
                        for nb in range(NB):         # 8 blocks of 512 tokens
                            tsl = slice(nb * 512, (nb + 1) * 512)
                            if nb >= 2:
                                load_ht(nb, [nc.sync])
                            ht_c = [ht_tiles[(nb, k)] for k in range(KC)]
                            if 1 <= nb <= 4:
                                # inv-rms broadcasts (DRAM->SBUF, partition
                                # bcast) trickled in behind cc1
                                for j in (2 * (nb - 1), 2 * nb - 1):
                                    sl = slice(j * 512, (j + 1) * 512)
                                    t = bcqp.tile([128, 512], F32, tag="bcq",
                                                  name=f"bcq{j}")
                                    nc.scalar.dma_start(
                                        t[:],
                                        cc1_out[j:j + 1, 0:TOKS]
                                        .to_broadcast((128, TOKS)))
                                    bcq_tiles[j] = t
                                    nc.scalar.dma_start(
                                        inv_kv_bc[:, sl],
                                        cc1_out[j:j + 1, TOKS:2 * TOKS]
                                        .to_broadcast((128, TOKS)))
                            ps_q = [pqps.tile([128, 512], F32, tag=f"pq{mb}",
                                              name=f"pq{mb}") for mb in range(3)]
                            for k in range(KC):
                                for mb in range(3):
                                    nc.tensor.matmul(
                                        ps_q[mb][:],
                                        lhsT=wq_t[k][:, mb * 128:(mb + 1) * 128],
                                        rhs=ht_c[k][:],
                                        start=(k == 0), stop=(k == KC - 1))
                            # evictions lag one block so the first broadcast
                            # (which waits on the gather) never stalls matmuls
                            if prev is not None:
                                evict_q(*prev)
                            prev = (nb, ps_q)
                        evict_q(*prev)

                    # gathered latents: emitted after all hT streaming so the
                    # cc2 dependency never blocks the sync queue's prefetches
                    kpe_raws = []
                    for r in range(NB):
                        for m in range(4):
                            nc.sync.dma_start(
                                kvg[m][:, r * 512:(r + 1) * 512],
                                cc2_out[r, m * 128 * TOKS:(m + 1) * 128 * TOKS])
                        kr = kpp.tile([64, 512], BF, tag="kpe_raw",
                                      name=f"kpe_raw{r}")
                        nc.gpsimd.dma_start(
                            kr[:], cc2_out[r, 4 * 128 * TOKS:CC2N])
                        kpe_raws.append(kr)

                    # ---- kv B-projection over all gathered tokens ----
                    with ExitStack() as pbs:
                        pbps = pbs.enter_context(
                            tc.tile_pool(name="pbps", bufs=2, space="PSUM"))
                        for r in range(NB):
                            rsl = slice(r * 512, (r + 1) * 512)
                            for h in range(HPC):
                                pkn = pbps.tile([128, 512], F32, tag="pkn",
                                                name="pkn")
                                for m in range(4):
                                    nc.tensor.matmul(
                                        pkn[:],
                                        lhsT=wkn_t[m][:, h * D_NOPE:
                                                      (h + 1) * D_NOPE],
                                        rhs=kvg[m][:, rsl],
                                        start=(m == 0), stop=(m == 3))
                                nc.vector.tensor_mul(kn[h][:, rsl], pkn[:],
                                                     inv_kv_bc[:, rsl])
                            # v for the 4 128-token chunks of this shard
                            for half in range(2):
                                pv = pbps.tile([128, 512], F32, tag="pv",
                                               name="pv")
                                for sub in range(2):
                                    tm = r * 4 + half * 2 + sub
                                    for m in range(4):
                                        nc.tensor.matmul(
                                            pv[:, sub * 256:(sub + 1) * 256],
                                            lhsT=kvg[m][:, tm * 128:
                                                        (tm + 1) * 128],
                                            rhs=wv_t[m][:],
                                            start=(m == 0 and sub == 0),
                                            stop=(m == 3 and sub == 1))
                                for sub in range(2):
                                    tm = r * 4 + half * 2 + sub
                                    nc.scalar.dma_start(
                                        ivc[tm][:],
                                        cc1_out[tm // 4, TOKS + (tm % 4) * 128:
                                                TOKS + (tm % 4) * 128 + 128])
                                    nc.scalar.mul(
                                        vnat[tm][:],
                                        pv[:, sub * 256:(sub + 1) * 256],
                                        ivc[tm][:])
                            # rope k_pe for this shard [E O]; csk1=[C S], csk2=[S C]
                            mka = p1sb.tile([32, 512], F32, tag="mka", name="mka")
                            mkb = p1sb.tile([32, 512], F32, tag="mkb", name="mkb")
                            nc.vector.tensor_mul(mka[:], kpe_raws[r][0:32, :],
                                                 csk1_t[0:32, rsl])
                            nc.vector.tensor_mul(mkb[:], kpe_raws[r][32:64, :],
                                                 csk1_t[32:64, rsl])
                            nc.vector.tensor_sub(kpe[0:32, rsl], mka[:], mkb[:])
                            mkc = p1sb.tile([32, 512], F32, tag="mka", name="mkc")
                            mkd = p1sb.tile([32, 512], F32, tag="mkb", name="mkd")
                            nc.vector.tensor_mul(mkc[:], kpe_raws[r][0:32, :],
                                                 csk2_t[0:32, rsl])
                            nc.vector.tensor_mul(mkd[:], kpe_raws[r][32:64, :],
                                                 csk2_t[32:64, rsl])
                            nc.vector.tensor_add(kpe[32:64, rsl], mkc[:], mkd[:])

                # ------------- phase 2+3: attention + wo -------------------
                if True:
                    wop = top.enter_context(tc.tile_pool(name="wop", bufs=1))
                    sps = top.enter_context(
                        tc.tile_pool(name="sps", bufs=1, space="PSUM"))
                    ops = top.enter_context(
                        tc.tile_pool(name="ops", bufs=2, space="PSUM"))
                    dps = top.enter_context(
                        tc.tile_pool(name="dps", bufs=1, space="PSUM"))
                    wps = top.enter_context(
                        tc.tile_pool(name="wps", bufs=1, space="PSUM"))
                    bnc = top.enter_context(
                        tc.tile_pool(name="bnc", bufs=4, space="DRAM"))
                    esb = top.enter_context(tc.tile_pool(name="esb", bufs=3))
                    asb = top.enter_context(tc.tile_pool(name="asb", bufs=2))
                    nsb = top.enter_context(tc.tile_pool(name="nsb", bufs=2))
                    otp = top.enter_context(tc.tile_pool(name="otp", bufs=2))
                    osb = top.enter_context(tc.tile_pool(name="osb", bufs=3))

                    wo_t = []
                    for i in range(2):
                        t = wop.tile([128, H], BF, tag=f"wot{i}", name=f"wot{i}")
                        nc.gpsimd.dma_start(t[:], woR[i * 128:(i + 1) * 128, :])
                        wo_t.append(t)

                    # deferred tensor-engine work, drained into slots of
                    # subsequent attention streams
                    pend = []

                    def drain_one():
                        if pend:
                            pend.pop(0)()

                    def make_den_head(ps_o, acc_d, outT_h, osl):
                        def emit():
                            ps_d = dps.tile([1, 512], F32, tag="ps_d", name="ps_d")
                            nc.tensor.matmul(
                                ps_d[:], lhsT=ones_col[:], rhs=acc_d[:],
                                start=True, stop=True)
                            rec = nsb.tile([1, 512], F32, tag="rec", name="rec")
                            nc.vector.reciprocal_approx_fast(rec[:], ps_d[:])
                            rec_d = bnc.tile([1, 512], F32, tag="rec_d",
                                             name="rec_d")
                            nc.sync.dma_start(rec_d[:], rec[:])
                            pend.insert(1, make_norm_tail(ps_o, rec_d, outT_h, osl))
                        return emit

                    def make_norm_tail(ps_o, rec_d, outT_h, osl):
                        def emit():
                            # broadcast 1/den across partitions via DRAM bounce
                            bc_sb = nsb.tile([128, 512], F32, tag="bc_sb",
                                             name="bc_sb")
                            nc.sync.dma_start(
                                bc_sb[:], rec_d[0:1, :].to_broadcast((128, 512)))
                            nc.vector.tensor_mul(outT_h[:, osl], ps_o[:], bc_sb[:])
                        return emit

                    def make_wo_group(outT, trow, osl128, hn, par):
                        def emit():
                            ps_w = wps.tile([128, 512], F32, tag="ps_w", name="ps_w")
                            for h in range(HPC):
                                nc.tensor.matmul(
                                    ps_w[:],
                                    lhsT=outT[h][:, osl128],
                                    rhs=wo_t[h][:, hn * 512:(hn + 1) * 512],
                                    start=(h == 0), stop=(h == HPC - 1))
                            o_sb = osb.tile([128, 512], BF, tag="o_sb", name="o_sb")
                            if par == 0:
                                nc.vector.tensor_copy(o_sb[:], ps_w[:])
                            else:
                                nc.scalar.copy(o_sb[:], ps_w[:])
                            nc.sync.dma_start(
                                out[trow:trow + 128, hn * 512:(hn + 1) * 512],
                                o_sb[:])
                        return emit

                    for b in range(B):
                        outT = [otp.tile([128, S], BF, tag=f"outT{h}", name=f"outT{h}")
                                for h in range(HPC)]
                        for qb in range(S // 512):
                            qsl = slice(b * S + qb * 512, b * S + qb * 512 + 512)
                            osl = slice(qb * 512, qb * 512 + 512)
                            ps_o = [ops.tile([128, 512], F32, tag=f"ps_o{h}",
                                             name=f"ps_o{h}") for h in range(HPC)]
                            acc_d = [asb.tile([128, 512], F32R, tag=f"acc{h}",
                                              name=f"acc{h}") for h in range(HPC)]
                            expp = [None, None]
                            for kc in range(S // 128):
                                ksl = slice(b * S + kc * 128,
                                            b * S + kc * 128 + 128)
                                expn = [None, None]
                                for h in range(HPC):
                                    ps_s = sps.tile([128, 512], F32, tag=f"ps_s{h}",
                                                    name=f"ps_s{h}")
                                    nc.tensor.matmul(ps_s[:], lhsT=kn[h][:, ksl],
                                                     rhs=qn[h][:, qsl],
                                                     start=True, stop=False)
                                    nc.tensor.matmul(
                                        ps_s[:], lhsT=kpe[:, ksl],
                                        rhs=qpe[h][:, qsl],
                                        start=False, stop=True)
                                    expT = esb.tile([128, 512], BF, tag=f"exp{h}",
                                                    name=f"exp{h}")
                                    nc.scalar.activation(expT[:], ps_s[:], AF.Exp,
                                                         scale=SCALE)
                                    if kc == 0:
                                        nc.vector.tensor_copy(acc_d[h][:], expT[:])
                                    else:
                                        nc.vector.tensor_add(acc_d[h][:],
                                                             acc_d[h][:], expT[:])
                                    expn[h] = expT
                                if kc >= 1:
                                    tm = (b * S) // 128 + kc - 1
                                    for h in range(HPC):
                                        nc.tensor.matmul(
                                            ps_o[h][:],
                                            lhsT=vnat[tm][:, h * D_V:(h + 1) * D_V],
                                            rhs=expp[h][:],
                                            start=(kc - 1 == 0), stop=False)
                                expp = expn
                                if kc >= 2:
                                    drain_one()
                                    if kc >= 9:
                                        drain_one()
                            tm = (b * S) // 128 + S // 128 - 1
                            for h in range(HPC):
                                nc.tensor.matmul(
                                    ps_o[h][:],
                                    lhsT=vnat[tm][:, h * D_V:(h + 1) * D_V],
                                    rhs=expp[h][:],
                                    start=False, stop=True)
                            for h in range(HPC):
                                pend.append(make_den_head(ps_o[h], acc_d[h],
                                                          outT[h], osl))
                            # queue wo for this (b, qb) once both heads are in
                            for j in range(4):
                                tmb = qb * 4 + j
                                osl128 = slice(tmb * 128, tmb * 128 + 128)
                                trow = b * S + tmb * 128
                                for hn in range(H // 512):
                                    pend.append(make_wo_group(
                                        outT, trow, osl128, hn, (j + hn) % 2))

                    while pend:
                        drain_one()

    nc.compile()
    return nc


def cc_in_slice(cc1_in, tb):
    return cc1_in[0, tb * 128:(tb + 1) * 128]


_PROGRAM = None


def _get_program():
    global _PROGRAM
    if _PROGRAM is None:
        _PROGRAM = _build_program()
    return _PROGRAM


def kernel(hidden_states, wq_a, q_norm_w, wq_b, wkv_a, kv_norm_w, wkv_b, wo):
    nc = _get_program()
    in_maps = _host_prep(hidden_states, wq_a, q_norm_w, wq_b,
                         wkv_a, kv_norm_w, wkv_b, wo)
    res = run_bass_kernel_spmd(nc, in_maps, list(range(N_CORES)))
    total = np.zeros((TOK, H), dtype=np.float32)
    for r in res.results:
        total += np.asarray(r["out"], dtype=np.float32)
    return total.reshape(B, S, H)
